# revision 13
# baseline (speedup 1.0000x reference)
"""EulerGCN on 8 trn2 NeuronCores — single SPMD launch.

Core t owns snapshot t for the GCN encode: 2 GCN props via ELL gathers +
DVE tree reduce + scatter-add into natural-order DRAM accumulators
(self-loops folded in as ordinary edge tokens; both props share one token
stream since the adjacency is identical). finish1 is a pure streaming
DVE pass; finish2 applies W2 per 128-node block via PE transpose+matmul
and emits tanh(emb)^T feature-major. An in-NEFF AllToAll reshards
feature-major slabs to node-parallel, then a transposed GRU + linear
head run in the same NEFF. Host does integer layout (edge grouping,
degree sort, token grids), GCN normalization, x@W1, and the final
output transpose.
"""

import sys
import time
import numpy as np
import ml_dtypes
import concourse.bass as bass
import concourse.bacc as bacc
import concourse.mybir as mybir
import concourse.tile as tile
from concourse.bass_utils import run_bass_kernel_spmd
from concourse.masks import make_identity

P = 128
NCORES = 8
N = 100000
NPAD = 100352           # 784 blocks of 128
QN = NPAD // 4          # 25088
QBLK = QN // P          # 196
NBLK = NPAD // P        # 784
T = 8
XD = 128
H = 64
Z = 32
NSH = NPAD // NCORES    # 12544
GCH = 448               # GRU chunk cols (28 * 448 = NSH)
NGCH = NSH // GCH
F32 = mybir.dt.float32
BF16 = mybir.dt.bfloat16
I16 = mybir.dt.int16
BF = ml_dtypes.bfloat16

PERF = {}


def _tick(label, t0):
    dt = time.time() - t0
    PERF[label] = PERF.get(label, 0.0) + dt
    print(f"[kernel] {label}: {dt:.2f}s", file=sys.stderr, flush=True)
    return time.time()


def wrap16(a):
    return np.ascontiguousarray(a.reshape(-1, 16).T)


def _prep_core(args):
    eis_c0, eis_c1, ews_c = args
    src = eis_c0.astype(np.int32)
    dst = eis_c1.astype(np.int32)
    w = ews_c.astype(np.float32)
    deg = np.bincount(dst, weights=w, minlength=N).astype(np.float32) + 1.0
    dinv = 1.0 / np.sqrt(deg)                                # [N]
    loops = np.arange(N, dtype=np.int32)
    src = np.concatenate([src, loops])
    dst = np.concatenate([dst, loops])
    wd = np.concatenate([w, np.ones(N, np.float32)]) * dinv[dst]

    dl = dst % QN
    sec8 = ((dst // QN) * 4 + (src // QN)).astype(np.int8)
    key0 = sec8.astype(np.int32) * QN + dl          # (section, local dst)
    cnt_all = np.bincount(key0, minlength=16 * QN)
    rank_all = np.empty(16 * QN, np.int16)
    orders, Ls_all = [], []
    for s in range(16):
        cnt = cnt_all[s * QN:(s + 1) * QN]
        order = np.argsort(-cnt, kind="stable")              # full QN perm
        rank_all[s * QN + order] = np.arange(QN, dtype=np.int16)
        orders.append(order.astype(np.int32))
        Ls_all.append(cnt[order].reshape(QBLK, P).max(axis=1).astype(np.int64))

    er_all = rank_all[key0]                          # int16, < QN
    # LSD radix: stable-sort by er (minor), then by section (major)
    o1 = np.argsort(er_all, kind="stable")
    o2 = np.argsort(sec8[o1], kind="stable")
    eo = o1[o2]
    er_sorted = er_all[eo]
    k1s = sec8[eo].astype(np.int32) * QN + er_sorted
    slot_all = (np.arange(k1s.size, dtype=np.int64)
                - np.searchsorted(k1s, k1s)).astype(np.int16)
    src_l = (src % QN).astype(np.int16)[eo]
    w_sorted = wd[eo]
    bounds = np.searchsorted(k1s, np.arange(17, dtype=np.int32) * QN)

    secs = []
    for s in range(16):
        lo, hi = bounds[s], bounds[s + 1]
        secs.append(dict(er=er_sorted[lo:hi].astype(np.int32),
                         slot=slot_all[lo:hi],
                         src=src_l[lo:hi], w=w_sorted[lo:hi],
                         order=orders[s], Ls=Ls_all[s]))
    return dict(dinv=dinv, secs=secs)


def build_host(x, eis, ews, W1):
    """Per-core tables and shared-shape token grids."""
    xw1 = x.astype(np.float32) @ W1.astype(np.float32)
    percore = [_prep_core((eis[c, 0], eis[c, 1], ews[c]))
               for c in range(T)]

    # common per-section block L (max over cores)
    commonL, nbs = [], []
    for s in range(16):
        Lc = np.zeros(QBLK, np.int64)
        for pc in percore:
            Lc = np.maximum(Lc, pc["secs"][s]["Ls"])
        nz = np.nonzero(Lc)[0]
        nb = int(nz[-1]) + 1 if nz.size else 1
        commonL.append(Lc[:nb])
        nbs.append(nb)
    sec_tok = [int(L.sum()) * P for L in commonL]
    sec_scat = [nb * P for nb in nbs]
    tok_total = sum(sec_tok)
    scat_total = sum(sec_scat)

    offs = [np.concatenate([[0], np.cumsum(Lc)]) * P for Lc in commonL]

    def _streams_core(c):
        g_all = np.zeros(tok_total, np.int16)
        w_all = np.zeros(tok_total, np.float32)
        s_all = np.empty(scat_total, np.int16)
        go = so = 0
        for s in range(16):
            ssec = percore[c]["secs"][s]
            off = offs[s]
            er, slot = ssec["er"], ssec["slot"]
            pos = off[er >> 7] + slot * P + (er & 127)
            g_all[go + pos] = ssec["src"].astype(np.int16)
            w_all[go + pos] = ssec["w"]
            s_all[so:so + sec_scat[s]] = ssec["order"][:sec_scat[s]].astype(np.int16)
            go += sec_tok[s]
            so += sec_scat[s]
        return dict(gidx=g_all, gw=w_all, sidx=s_all)

    streams = [_streams_core(c) for c in range(T)]

    xw1p = np.zeros((NPAD, H), np.float32)
    xw1p[:N] = xw1
    xw1bf = xw1p.astype(BF)
    tables = []
    for c in range(T):
        dpad = np.zeros(NPAD, np.float32)
        dpad[:N] = percore[c]["dinv"]
        tables.append(dict(dinv_blk=dpad.reshape(NBLK, P).T.copy()))
    return dict(commonL=commonL, nbs=nbs, sec_tok=sec_tok, sec_scat=sec_scat,
                tok_total=tok_total, scat_total=scat_total,
                streams=streams, tables=tables, xw1bf=xw1bf)


def build_program(hp):
    commonL = hp["commonL"]
    sec_tok = hp["sec_tok"]
    sec_scat = hp["sec_scat"]
    tok_total = hp["tok_total"]
    scat_total = hp["scat_total"]
    max_tok = max(sec_tok)
    max_scat = max(sec_scat)

    nc = bacc.Bacc(trn_type="TRN2", num_devices=NCORES, num_swdge_queues=4)
    t1bf_d = nc.dram_tensor("xw1bf", [NPAD, H], BF16, kind="ExternalInput")
    gidx_d = nc.dram_tensor("gidx16", [16, tok_total // 16], I16, kind="ExternalInput")
    gw_d = nc.dram_tensor("gw128", [P, tok_total // P], BF16, kind="ExternalInput")
    sidx_d = nc.dram_tensor("sidx16", [16, scat_total // 16], I16, kind="ExternalInput")
    dinv_d = nc.dram_tensor("dinv_blk", [P, NBLK], F32, kind="ExternalInput")
    b1b_d = nc.dram_tensor("b1b", [P, H], F32, kind="ExternalInput")
    b2c_d = nc.dram_tensor("b2c", [H, 1], F32, kind="ExternalInput")
    W2_d = nc.dram_tensor("W2", [H, H], F32, kind="ExternalInput")
    wihT_d = nc.dram_tensor("wihT", [H, 3 * H], BF16, kind="ExternalInput")
    whhT_d = nc.dram_tensor("whhT", [H, 3 * H], BF16, kind="ExternalInput")
    wlinT_d = nc.dram_tensor("wlinT", [H, Z], BF16, kind="ExternalInput")
    br_d = nc.dram_tensor("br", [H, 1], F32, kind="ExternalInput")
    bz_d = nc.dram_tensor("bz", [H, 1], F32, kind="ExternalInput")
    bin_d = nc.dram_tensor("bin", [H, 1], F32, kind="ExternalInput")
    bhn_d = nc.dram_tensor("bhn", [H, 1], F32, kind="ExternalInput")
    blin_d = nc.dram_tensor("blin", [Z, 1], F32, kind="ExternalInput")
    ysT_d = nc.dram_tensor("ysT", [T, Z, NSH], BF16, kind="ExternalOutput")

    table1 = nc.dram_tensor("table1", [NPAD, H], F32)
    table2 = nc.dram_tensor("table2", [NPAD, H], F32)
    acc = [nc.dram_tensor(f"acc{pr}", [NPAD, H], F32) for pr in range(2)]

    with tile.TileContext(nc) as tc:
        with tc.tile_pool(name="const", bufs=1) as cpool, \
             tc.tile_pool(name="dram", bufs=1, space="DRAM") as dpool:
            ident = cpool.tile([P, P], F32)
            make_identity(nc, ident[:])
            dinv_t = cpool.tile([P, NBLK], F32)
            b1_t = cpool.tile([P, H], F32)
            b2c_t = cpool.tile([H, 1], F32)
            W2_t = cpool.tile([H, H], F32)
            wih_t = cpool.tile([H, 3 * H], BF16)
            whh_t = cpool.tile([H, 3 * H], BF16)
            wlin_t = cpool.tile([H, Z], BF16)
            br_t = cpool.tile([H, 1], F32)
            bz_t = cpool.tile([H, 1], F32)
            bin_t = cpool.tile([H, 1], F32)
            bhn_t = cpool.tile([H, 1], F32)
            blin_t = cpool.tile([Z, 1], F32)
            for tt, dd in ((dinv_t, dinv_d), (b1_t, b1b_d), (b2c_t, b2c_d),
                           (W2_t, W2_d), (wih_t, wihT_d), (whh_t, whhT_d),
                           (wlin_t, wlinT_d), (br_t, br_d), (bz_t, bz_d),
                           (bin_t, bin_d), (bhn_t, bhn_d), (blin_t, blin_d)):
                nc.sync.dma_start(out=tt[:], in_=dd[:])

            cc_in = dpool.tile([NCORES * H, NSH], BF16)
            cc_out = dpool.tile([NCORES * H, NSH], BF16)

            # zero accumulators (32 x 0.8MB DMAs)
            zt = cpool.tile([P, 1568], F32)
            nc.gpsimd.memset(zt[:], 0.0)
            for pr in range(2):
                for a0 in range(0, NPAD, 3136):
                    nc.sync.dma_start(out=acc[pr][a0:a0 + 3136, :],
                                      in_=zt[:])

            # expand table1 bf16 -> f32 (8 chunks of 98 blocks)
            with tc.tile_pool(name="exp", bufs=2) as epool:
                for k in range(0, NBLK, 98):
                    src = t1bf_d[k * P:(k + 98) * P, :].rearrange(
                        "(j p) h -> p j h", p=P)
                    tb = epool.tile([P, 98, H], BF16, tag="tbf")
                    nc.sync.dma_start(out=tb[:], in_=src)
                    tf = epool.tile([P, 98, H], F32, tag="tf32")
                    nc.vector.tensor_copy(out=tf[:], in_=tb[:])
                    nc.vector.tensor_tensor(
                        out=tf[:], in0=tf[:],
                        in1=dinv_t[:, k:k + 98].unsqueeze(-1)
                            .broadcast_to([P, 98, H]),
                        op=mybir.AluOpType.mult)
                    nc.sync.dma_start(
                        out=table1[k * P:(k + 98) * P, :].rearrange(
                            "(j p) h -> p j h", p=P),
                        in_=tf[:])

            # ---- the two props ----
            with tc.tile_pool(name="sec", bufs=2) as spool, \
                 tc.tile_pool(name="gath", bufs=3) as gpool:
                qcount = 0
                for pr in range(2):
                    table = table1 if pr == 0 else table2
                    go = so = 0
                    for s in range(16):
                        r, q = divmod(s, 4)
                        Lc = commonL[s]
                        stok, ssc = sec_tok[s], sec_scat[s]
                        if stok == 0:
                            go += stok
                            so += ssc
                            continue
                        gi_b = spool.tile([P, max_tok // 16], I16, tag="gi")
                        si_b = spool.tile([P, max_scat // 16], I16, tag="si")
                        for k in range(8):
                            nc.sync.dma_start(
                                out=gi_b[16 * k:16 * k + 16, :stok // 16],
                                in_=gidx_d[:, go // 16:(go + stok) // 16])
                            nc.sync.dma_start(
                                out=si_b[16 * k:16 * k + 16, :ssc // 16],
                                in_=sidx_d[:, so // 16:(so + ssc) // 16])
                        wbf = spool.tile([P, max_tok // P], BF16, tag="wbf")
                        nc.sync.dma_start(out=wbf[:, :stok // P],
                                          in_=gw_d[:, go // P:(go + stok) // P])
                        w_b = spool.tile([P, max_tok // P], F32, tag="wf")
                        nc.vector.tensor_copy(out=w_b[:, :stok // P],
                                              in_=wbf[:, :stok // P])

                        tbl = table[q * QN:(q + 1) * QN, :]
                        accr = acc[pr][r * QN:(r + 1) * QN, :]
                        lgo = lso = 0   # local token / scatter offsets
                        b = 0
                        while b < len(Lc):
                            L = int(Lc[b])
                            b2 = b
                            while b2 < len(Lc) and int(Lc[b2]) == L:
                                b2 += 1
                            if L == 0:
                                b = b2
                                continue
                            assert L <= 64, L
                            gpc = max(1, 64 // L)
                            bb = b
                            while bb < b2:
                                nbb = min(gpc, b2 - bb)
                                ncols = nbb * L
                                tok = ncols * P
                                stk = nbb * P
                                pk = gpool.tile([P, 64, H], F32, tag="pk")
                                if L == 1:
                                    gt = gpool.tile([P, 64, H], F32, tag="g")
                                    nc.gpsimd.dma_gather(
                                        out_ap=gt[:, :ncols, :], in_ap=tbl,
                                        idxs_ap=gi_b[:, lgo // 16:(lgo + tok) // 16],
                                        num_idxs=tok, num_idxs_reg=tok,
                                        elem_size=H, single_packet=False,
                                        queue_num=qcount % 4)
                                    nc.vector.tensor_tensor(
                                        out=pk[:, :ncols, :], in0=gt[:, :ncols, :],
                                        in1=w_b[:, lgo // P:lgo // P + ncols]
                                            .unsqueeze(-1)
                                            .broadcast_to([P, ncols, H]),
                                        op=mybir.AluOpType.mult)
                                else:
                                    gt = gpool.tile([P, 64, H], F32, tag="g")
                                    nc.gpsimd.dma_gather(
                                        out_ap=gt[:, :ncols, :], in_ap=tbl,
                                        idxs_ap=gi_b[:, lgo // 16:(lgo + tok) // 16],
                                        num_idxs=tok, num_idxs_reg=tok,
                                        elem_size=H, single_packet=False,
                                        queue_num=qcount % 4)
                                    nc.vector.tensor_tensor(
                                        out=gt[:, :ncols, :], in0=gt[:, :ncols, :],
                                        in1=w_b[:, lgo // P:lgo // P + ncols]
                                            .unsqueeze(-1)
                                            .broadcast_to([P, ncols, H]),
                                        op=mybir.AluOpType.mult)
                                    gv = gt[:, :ncols, :].rearrange(
                                        "p (g l) h -> p g l h", l=L)
                                    width = L
                                    while width > 2:
                                        half = width // 2
                                        nc.vector.tensor_tensor(
                                            out=gv[:, :, :half, :],
                                            in0=gv[:, :, :half, :],
                                            in1=gv[:, :, width - half:width, :],
                                            op=mybir.AluOpType.add)
                                        width -= half
                                    if width == 2:
                                        nc.vector.tensor_tensor(
                                            out=pk[:, :nbb, :],
                                            in0=gv[:, :, 0, :], in1=gv[:, :, 1, :],
                                            op=mybir.AluOpType.add)
                                    else:
                                        nc.vector.tensor_copy(
                                            out=pk[:, :nbb, :], in_=gv[:, :, 0, :])
                                nc.gpsimd.dma_scatter_add(
                                    accr, pk[:, :nbb, :],
                                    si_b[:, lso // 16:(lso + stk) // 16],
                                    stk, stk, H, queue_num=qcount % 4)
                                qcount += 1
                                lgo += tok
                                lso += stk
                                bb += nbb
                            b = b2
                        go += stok
                        so += ssc

                    # ---- finish pass ----
                    if pr == 0:
                        with tc.tile_pool(name="fin", bufs=2) as fpool:
                            for k in range(0, NBLK, 49):
                                av = fpool.tile([P, 49, H], F32, tag="av")
                                nc.sync.dma_start(
                                    out=av[:],
                                    in_=acc[0][k * P:(k + 49) * P, :].rearrange(
                                        "(j p) h -> p j h", p=P))
                                nc.vector.tensor_tensor(
                                    out=av[:], in0=av[:],
                                    in1=b1_t[:].unsqueeze(1)
                                        .broadcast_to([P, 49, H]),
                                    op=mybir.AluOpType.add)
                                nc.vector.tensor_scalar_max(
                                    out=av[:], in0=av[:], scalar1=0.0)
                                nc.vector.tensor_tensor(
                                    out=av[:], in0=av[:],
                                    in1=dinv_t[:, k:k + 49].unsqueeze(-1)
                                        .broadcast_to([P, 49, H]),
                                    op=mybir.AluOpType.mult)
                                nc.sync.dma_start(
                                    out=table2[k * P:(k + 49) * P, :].rearrange(
                                        "(j p) h -> p j h", p=P),
                                    in_=av[:])

            # ---- finish2: W2, bias, tanh, transpose to feature-major ----
            with tc.tile_pool(name="f2", bufs=3) as f2pool, \
                 tc.tile_pool(name="f2p", bufs=4, space="PSUM") as f2ps:
                for j in range(NCORES):          # peer slab
                    for c0 in range(0, 98, 8):
                        nb2 = min(8, 98 - c0)
                        k0 = j * 98 + c0
                        av2 = f2pool.tile([P, 8, H], F32, tag="av2")
                        nc.sync.dma_start(
                            out=av2[:, :nb2, :],
                            in_=acc[1][k0 * P:(k0 + nb2) * P, :].rearrange(
                                "(j p) h -> p j h", p=P))
                        for g0 in range(0, nb2, 4):
                            ng = min(4, nb2 - g0)
                            eg = f2pool.tile([H, 4 * P], BF16, tag="eg")
                            for bi in range(ng):
                                pt = f2ps.tile([H, P], F32, tag="pt")
                                nc.tensor.transpose(
                                    out=pt[:], in_=av2[:, g0 + bi, :],
                                    identity=ident[:])
                                abT = f2pool.tile([H, P], F32, tag="abT")
                                nc.vector.tensor_copy(out=abT[:], in_=pt[:])
                                mm = f2ps.tile([H, P], F32, tag="mm")
                                nc.tensor.matmul(out=mm[:], lhsT=W2_t[:],
                                                 rhs=abT[:], start=True, stop=True)
                                nc.scalar.activation(
                                    out=eg[:, bi * P:(bi + 1) * P], in_=mm[:],
                                    func=mybir.ActivationFunctionType.Tanh,
                                    bias=b2c_t[:])
                            cz = (c0 + g0) * P
                            nc.sync.dma_start(
                                out=cc_in[j * H:(j + 1) * H, cz:cz + ng * P],
                                in_=eg[:, :ng * P])

            # ---- AllToAll reshard ----
            nc.gpsimd.collective_compute(
                "AllToAll", mybir.AluOpType.bypass,
                replica_groups=[list(range(NCORES))],
                ins=[cc_in[:]], outs=[cc_out[:]])

            # ---- GRU + head (transposed layout) ----
            with tc.tile_pool(name="gs", bufs=1) as gspool, \
                 tc.tile_pool(name="gx", bufs=2) as gxpool, \
                 tc.tile_pool(name="gw", bufs=2) as gwpool, \
                 tc.tile_pool(name="gp", bufs=2, space="PSUM") as gppool, \
                 tc.tile_pool(name="gp1", bufs=1, space="PSUM") as gppool1:
                h32 = gspool.tile([H, NSH], F32)
                nc.gpsimd.memset(h32[:], 0.0)
                for t in range(T):
                    xsT = gxpool.tile([H, NSH], BF16, tag="xs")
                    nc.sync.dma_start(out=xsT[:],
                                      in_=cc_out[t * H:(t + 1) * H, :])
                    y_t = gxpool.tile([Z, NSH], BF16, tag="y")
                    for i in range(NGCH):
                        sl = slice(i * GCH, (i + 1) * GCH)
                        hb = gwpool.tile([H, GCH], BF16, tag="hb")
                        nc.vector.tensor_copy(out=hb[:], in_=h32[:, sl])
                        mm_r = gppool.tile([H, GCH], F32, tag="mr")
                        nc.tensor.matmul(out=mm_r[:], lhsT=wih_t[:, :H],
                                         rhs=xsT[:, sl], start=True, stop=False)
                        nc.tensor.matmul(out=mm_r[:], lhsT=whh_t[:, :H],
                                         rhs=hb[:], start=False, stop=True)
                        mm_z = gppool.tile([H, GCH], F32, tag="mz")
                        nc.tensor.matmul(out=mm_z[:], lhsT=wih_t[:, H:P],
                                         rhs=xsT[:, sl], start=True, stop=False)
                        nc.tensor.matmul(out=mm_z[:], lhsT=whh_t[:, H:P],
                                         rhs=hb[:], start=False, stop=True)
                        r_sb = gwpool.tile([H, GCH], F32, tag="r")
                        nc.scalar.activation(
                            out=r_sb[:], in_=mm_r[:],
                            func=mybir.ActivationFunctionType.Sigmoid,
                            bias=br_t[:])
                        z_sb = gwpool.tile([H, GCH], F32, tag="z")
                        nc.scalar.activation(
                            out=z_sb[:], in_=mm_z[:],
                            func=mybir.ActivationFunctionType.Sigmoid,
                            bias=bz_t[:])
                        mm_hn = gppool1.tile([H, GCH], F32, tag="mhn")
                        nc.tensor.matmul(out=mm_hn[:], lhsT=whh_t[:, P:],
                                         rhs=hb[:], start=True, stop=True)
                        rn = gwpool.tile([H, GCH], F32, tag="rn")
                        nc.vector.tensor_scalar_add(
                            out=rn[:], in0=mm_hn[:], scalar1=bhn_t[:])
                        nc.vector.tensor_tensor(
                            out=rn[:], in0=rn[:], in1=r_sb[:],
                            op=mybir.AluOpType.mult)
                        mm_in = gppool1.tile([H, GCH], F32, tag="min")
                        nc.tensor.matmul(out=mm_in[:], lhsT=wih_t[:, P:],
                                         rhs=xsT[:, sl], start=True, stop=True)
                        npre = gwpool.tile([H, GCH], F32, tag="npre")
                        nc.vector.tensor_tensor(
                            out=npre[:], in0=mm_in[:], in1=rn[:],
                            op=mybir.AluOpType.add)
                        n_sb = gwpool.tile([H, GCH], F32, tag="nsb")
                        nc.scalar.activation(
                            out=n_sb[:], in_=npre[:],
                            func=mybir.ActivationFunctionType.Tanh,
                            bias=bin_t[:])
                        d = gwpool.tile([H, GCH], F32, tag="d")
                        nc.vector.tensor_tensor(
                            out=d[:], in0=h32[:, sl], in1=n_sb[:],
                            op=mybir.AluOpType.subtract)
                        nc.vector.tensor_tensor(
                            out=d[:], in0=d[:], in1=z_sb[:],
                            op=mybir.AluOpType.mult)
                        nc.vector.tensor_tensor(
                            out=h32[:, sl], in0=n_sb[:], in1=d[:],
                            op=mybir.AluOpType.add)
                        hb2 = gwpool.tile([H, GCH], BF16, tag="hb2")
                        nc.vector.tensor_copy(out=hb2[:], in_=h32[:, sl])
                        mm_y = gppool.tile([Z, GCH], F32, tag="my")
                        nc.tensor.matmul(out=mm_y[:], lhsT=wlin_t[:],
                                         rhs=hb2[:], start=True, stop=True)
                        nc.vector.tensor_scalar_add(
                            out=y_t[:, sl], in0=mm_y[:], scalar1=blin_t[:])
                    nc.sync.dma_start(out=ysT_d[t], in_=y_t[:])
    nc.compile()
    return nc


def _warm_devices():
    try:
        import jax
        from jax.sharding import Mesh, PartitionSpec, NamedSharding
        devs = jax.devices()[:NCORES]
        mesh = Mesh(np.asarray(devs), ("core",))
        sh = NamedSharding(mesh, PartitionSpec("core"))
        jax.device_put(np.zeros((NCORES, 4), np.float32), sh).block_until_ready()
    except Exception as e:
        print(f"[kernel] device warm-up failed: {e}", file=sys.stderr)


def _start_warm():
    try:
        import threading
        import jax
        jax.devices()      # backend init on the importing thread
        th = threading.Thread(target=_warm_devices, daemon=True)
        th.start()
        return th
    except Exception as e:
        print(f"[kernel] warm start failed: {e}", file=sys.stderr)
        return None


_WARM = _start_warm()


def kernel(**inputs):
    warm = _WARM if _WARM is not None else _start_warm()
    x = np.asarray(inputs["x"], np.float32)
    eis = np.asarray(inputs["eis"])
    ews = np.asarray(inputs["ews"], np.float32)
    W1 = np.asarray(inputs["W1"], np.float32)
    b1 = np.asarray(inputs["b1"], np.float32)
    W2 = np.asarray(inputs["W2"], np.float32)
    b2 = np.asarray(inputs["b2"], np.float32)
    Wih = np.asarray(inputs["Wih"], np.float32)
    Whh = np.asarray(inputs["Whh"], np.float32)
    bih = np.asarray(inputs["bih"], np.float32)
    bhh = np.asarray(inputs["bhh"], np.float32)
    Wlin = np.asarray(inputs["Wlin"], np.float32)
    blin = np.asarray(inputs["blin"], np.float32)

    _t0 = time.time()
    hp = build_host(x, eis, ews, W1)
    _t0 = _tick("host-prep", _t0)

    nc = build_program(hp)
    _t0 = _tick("build", _t0)

    b1b = np.broadcast_to(b1, (P, H)).copy()
    b2c = b2.reshape(H, 1).copy()
    brc = (bih[:H] + bhh[:H]).reshape(H, 1).copy()
    bzc = (bih[H:2 * H] + bhh[H:2 * H]).reshape(H, 1).copy()
    binc = bih[2 * H:].reshape(H, 1).copy()
    bhnc = bhh[2 * H:].reshape(H, 1).copy()
    blinc = blin.reshape(Z, 1).copy()
    wihT = np.ascontiguousarray(Wih.T).astype(BF)
    whhT = np.ascontiguousarray(Whh.T).astype(BF)
    wlinT = np.ascontiguousarray(Wlin.T).astype(BF)

    in_maps = []
    for c in range(NCORES):
        st = hp["streams"][c]
        tb = hp["tables"][c]
        in_maps.append({
            "xw1bf": hp["xw1bf"],
            "gidx16": wrap16(st["gidx"]),
            "gw128": np.ascontiguousarray(
                st["gw"].reshape(-1, P).T).astype(BF),
            "sidx16": wrap16(st["sidx"]),
            "dinv_blk": tb["dinv_blk"],
            "b1b": b1b, "b2c": b2c, "W2": W2,
            "wihT": wihT, "whhT": whhT, "wlinT": wlinT,
            "br": brc, "bz": bzc, "bin": binc, "bhn": bhnc, "blin": blinc,
        })
    _t0 = _tick("inmaps", _t0)
    if warm is not None:
        warm.join()
    _t0 = _tick("warm-join", _t0)

    res = run_bass_kernel_spmd(nc, in_maps, core_ids=list(range(NCORES)))
    _t0 = _tick("run", _t0)

    out = np.empty((T, N, Z), np.float32)
    for c in range(NCORES):
        lo, hi = c * NSH, min((c + 1) * NSH, N)
        if lo >= N:
            continue
        ys = np.asarray(res.results[c]["ysT"], dtype=np.float32)  # [T, Z, NSH]
        out[:, lo:hi, :] = ys.transpose(0, 2, 1)[:, :hi - lo, :]
    _t0 = _tick("assemble", _t0)
    return out


# revision 14
# speedup vs baseline: 1.4720x; 1.4720x over previous
"""EulerGCN on 8 trn2 NeuronCores — single SPMD launch.

Core t owns snapshot t for the GCN encode: 2 GCN props via ELL gathers +
DVE tree reduce + scatter-add into natural-order DRAM accumulators
(self-loops folded in as ordinary edge tokens; both props share one token
stream since the adjacency is identical). finish1 is a pure streaming
DVE pass; finish2 applies W2 per 128-node block via PE transpose+matmul
and emits tanh(emb)^T feature-major. An in-NEFF AllToAll reshards
feature-major slabs to node-parallel, then a transposed GRU + linear
head run in the same NEFF. Host does integer layout (edge grouping,
degree sort, token grids), GCN normalization, x@W1, and the final
output transpose.
"""

import sys
import time
import numpy as np
import ml_dtypes
import concourse.bass as bass
import concourse.bacc as bacc
import concourse.mybir as mybir
import concourse.tile as tile
from concourse.bass_utils import run_bass_kernel_spmd
from concourse.masks import make_identity

P = 128
NCORES = 8
N = 100000
NPAD = 100352           # 784 blocks of 128
QN = NPAD // 4          # 25088
QBLK = QN // P          # 196
NBLK = NPAD // P        # 784
T = 8
XD = 128
H = 64
Z = 32
NSH = NPAD // NCORES    # 12544
GCH = 448               # GRU chunk cols (28 * 448 = NSH)
NGCH = NSH // GCH
F32 = mybir.dt.float32
BF16 = mybir.dt.bfloat16
I16 = mybir.dt.int16
BF = ml_dtypes.bfloat16

PERF = {}


def _tick(label, t0):
    dt = time.time() - t0
    PERF[label] = PERF.get(label, 0.0) + dt
    print(f"[kernel] {label}: {dt:.2f}s", file=sys.stderr, flush=True)
    return time.time()


def wrap16(a):
    return np.ascontiguousarray(a.reshape(-1, 16).T)


def _prep_core(args):
    eis_c0, eis_c1, ews_c = args
    src = eis_c0.astype(np.int32)
    dst = eis_c1.astype(np.int32)
    w = ews_c.astype(np.float32)
    deg = np.bincount(dst, weights=w, minlength=N).astype(np.float32) + 1.0
    dinv = 1.0 / np.sqrt(deg)                                # [N]
    loops = np.arange(N, dtype=np.int32)
    src = np.concatenate([src, loops])
    dst = np.concatenate([dst, loops])
    wd = np.concatenate([w, np.ones(N, np.float32)]) * dinv[dst]

    dl = dst % QN
    sec8 = ((dst // QN) * 4 + (src // QN)).astype(np.int8)
    key0 = sec8.astype(np.int32) * QN + dl          # (section, local dst)
    cnt_all = np.bincount(key0, minlength=16 * QN)
    rank_all = np.empty(16 * QN, np.int16)
    orders, Ls_all = [], []
    for s in range(16):
        cnt = cnt_all[s * QN:(s + 1) * QN]
        order = np.argsort(-cnt, kind="stable")              # full QN perm
        rank_all[s * QN + order] = np.arange(QN, dtype=np.int16)
        orders.append(order.astype(np.int32))
        Ls_all.append(cnt[order].reshape(QBLK, P).max(axis=1).astype(np.int64))

    er_all = rank_all[key0]                          # int16, < QN
    # LSD radix: stable-sort by er (minor), then by section (major)
    o1 = np.argsort(er_all, kind="stable")
    o2 = np.argsort(sec8[o1], kind="stable")
    eo = o1[o2]
    er_sorted = er_all[eo]
    k1s = sec8[eo].astype(np.int32) * QN + er_sorted
    slot_all = (np.arange(k1s.size, dtype=np.int64)
                - np.searchsorted(k1s, k1s)).astype(np.int16)
    src_l = (src % QN).astype(np.int16)[eo]
    w_sorted = wd[eo]
    bounds = np.searchsorted(k1s, np.arange(17, dtype=np.int32) * QN)

    secs = []
    for s in range(16):
        lo, hi = bounds[s], bounds[s + 1]
        secs.append(dict(er=er_sorted[lo:hi].astype(np.int32),
                         slot=slot_all[lo:hi],
                         src=src_l[lo:hi], w=w_sorted[lo:hi],
                         order=orders[s], Ls=Ls_all[s]))
    return dict(dinv=dinv, secs=secs)


def build_host(x, eis, ews, W1):
    """Per-core tables and shared-shape token grids."""
    xw1 = x.astype(np.float32) @ W1.astype(np.float32)
    percore = [_prep_core((eis[c, 0], eis[c, 1], ews[c]))
               for c in range(T)]

    # common per-section block L (max over cores)
    commonL, nbs = [], []
    for s in range(16):
        Lc = np.zeros(QBLK, np.int64)
        for pc in percore:
            Lc = np.maximum(Lc, pc["secs"][s]["Ls"])
        nz = np.nonzero(Lc)[0]
        nb = int(nz[-1]) + 1 if nz.size else 1
        commonL.append(Lc[:nb])
        nbs.append(nb)
    sec_tok = [int(L.sum()) * P for L in commonL]
    sec_scat = [nb * P for nb in nbs]
    tok_total = sum(sec_tok)
    scat_total = sum(sec_scat)

    offs = [np.concatenate([[0], np.cumsum(Lc)]) * P for Lc in commonL]

    def _streams_core(c):
        g_all = np.zeros(tok_total, np.int16)
        w_all = np.zeros(tok_total, np.float32)
        s_all = np.empty(scat_total, np.int16)
        go = so = 0
        for s in range(16):
            ssec = percore[c]["secs"][s]
            off = offs[s]
            er, slot = ssec["er"], ssec["slot"]
            pos = off[er >> 7] + slot * P + (er & 127)
            g_all[go + pos] = ssec["src"].astype(np.int16)
            w_all[go + pos] = ssec["w"]
            s_all[so:so + sec_scat[s]] = ssec["order"][:sec_scat[s]].astype(np.int16)
            go += sec_tok[s]
            so += sec_scat[s]
        return dict(gidx=g_all, gw=w_all, sidx=s_all)

    streams = [_streams_core(c) for c in range(T)]

    xw1p = np.zeros((NPAD, H), np.float32)
    xw1p[:N] = xw1
    xw1bf = xw1p.astype(BF)
    tables = []
    for c in range(T):
        dpad = np.zeros(NPAD, np.float32)
        dpad[:N] = percore[c]["dinv"]
        tables.append(dict(dinv_blk=dpad.reshape(NBLK, P).T.copy()))
    return dict(commonL=commonL, nbs=nbs, sec_tok=sec_tok, sec_scat=sec_scat,
                tok_total=tok_total, scat_total=scat_total,
                streams=streams, tables=tables, xw1bf=xw1bf)


def build_program(hp):
    commonL = hp["commonL"]
    sec_tok = hp["sec_tok"]
    sec_scat = hp["sec_scat"]
    tok_total = hp["tok_total"]
    scat_total = hp["scat_total"]
    max_tok = max(sec_tok)
    max_scat = max(sec_scat)

    nc = bacc.Bacc(trn_type="TRN2", num_devices=NCORES, num_swdge_queues=4)
    t1bf_d = nc.dram_tensor("xw1bf", [NPAD, H], BF16, kind="ExternalInput")
    gidx_d = nc.dram_tensor("gidx16", [16, tok_total // 16], I16, kind="ExternalInput")
    gw_d = nc.dram_tensor("gw128", [P, tok_total // P], BF16, kind="ExternalInput")
    sidx_d = nc.dram_tensor("sidx16", [16, scat_total // 16], I16, kind="ExternalInput")
    dinv_d = nc.dram_tensor("dinv_blk", [P, NBLK], F32, kind="ExternalInput")
    b1b_d = nc.dram_tensor("b1b", [P, H], F32, kind="ExternalInput")
    b2c_d = nc.dram_tensor("b2c", [H, 1], F32, kind="ExternalInput")
    W2_d = nc.dram_tensor("W2", [H, H], F32, kind="ExternalInput")
    wihT_d = nc.dram_tensor("wihT", [H, 3 * H], BF16, kind="ExternalInput")
    whhT_d = nc.dram_tensor("whhT", [H, 3 * H], BF16, kind="ExternalInput")
    wlinT_d = nc.dram_tensor("wlinT", [H, Z], BF16, kind="ExternalInput")
    br_d = nc.dram_tensor("br", [H, 1], F32, kind="ExternalInput")
    bz_d = nc.dram_tensor("bz", [H, 1], F32, kind="ExternalInput")
    bin_d = nc.dram_tensor("bin", [H, 1], F32, kind="ExternalInput")
    bhn_d = nc.dram_tensor("bhn", [H, 1], F32, kind="ExternalInput")
    blin_d = nc.dram_tensor("blin", [Z, 1], F32, kind="ExternalInput")
    ysT_d = nc.dram_tensor("ysT", [T, Z, NSH], BF16, kind="ExternalOutput")

    table1 = nc.dram_tensor("table1", [NPAD, H], F32)
    table2 = nc.dram_tensor("table2", [NPAD, H], F32)
    acc = [nc.dram_tensor(f"acc{pr}", [NPAD, H], F32) for pr in range(2)]

    with tile.TileContext(nc) as tc:
        with tc.tile_pool(name="const", bufs=1) as cpool, \
             tc.tile_pool(name="dram", bufs=1, space="DRAM") as dpool:
            ident = cpool.tile([P, P], F32)
            make_identity(nc, ident[:])
            dinv_t = cpool.tile([P, NBLK], F32)
            b1_t = cpool.tile([P, H], F32)
            b2c_t = cpool.tile([H, 1], F32)
            W2_t = cpool.tile([H, H], F32)
            wih_t = cpool.tile([H, 3 * H], BF16)
            whh_t = cpool.tile([H, 3 * H], BF16)
            wlin_t = cpool.tile([H, Z], BF16)
            br_t = cpool.tile([H, 1], F32)
            bz_t = cpool.tile([H, 1], F32)
            bin_t = cpool.tile([H, 1], F32)
            bhn_t = cpool.tile([H, 1], F32)
            blin_t = cpool.tile([Z, 1], F32)
            for tt, dd in ((dinv_t, dinv_d), (b1_t, b1b_d), (b2c_t, b2c_d),
                           (W2_t, W2_d), (wih_t, wihT_d), (whh_t, whhT_d),
                           (wlin_t, wlinT_d), (br_t, br_d), (bz_t, bz_d),
                           (bin_t, bin_d), (bhn_t, bhn_d), (blin_t, blin_d)):
                nc.sync.dma_start(out=tt[:], in_=dd[:])

            cc_in = dpool.tile([NCORES * H, NSH], BF16)
            cc_out = dpool.tile([NCORES * H, NSH], BF16)

            # zero accumulators (32 x 0.8MB DMAs)
            zt = cpool.tile([P, 1568], F32)
            nc.gpsimd.memset(zt[:], 0.0)
            for pr in range(2):
                for a0 in range(0, NPAD, 3136):
                    nc.sync.dma_start(out=acc[pr][a0:a0 + 3136, :],
                                      in_=zt[:])

            # expand table1 bf16 -> f32 (8 chunks of 98 blocks)
            with tc.tile_pool(name="exp", bufs=2) as epool:
                for k in range(0, NBLK, 98):
                    src = t1bf_d[k * P:(k + 98) * P, :].rearrange(
                        "(j p) h -> p j h", p=P)
                    tb = epool.tile([P, 98, H], BF16, tag="tbf")
                    nc.sync.dma_start(out=tb[:], in_=src)
                    tf = epool.tile([P, 98, H], F32, tag="tf32")
                    nc.vector.tensor_copy(out=tf[:], in_=tb[:])
                    nc.vector.tensor_tensor(
                        out=tf[:], in0=tf[:],
                        in1=dinv_t[:, k:k + 98].unsqueeze(-1)
                            .broadcast_to([P, 98, H]),
                        op=mybir.AluOpType.mult)
                    nc.sync.dma_start(
                        out=table1[k * P:(k + 98) * P, :].rearrange(
                            "(j p) h -> p j h", p=P),
                        in_=tf[:])

            # ---- the two props ----
            with tc.tile_pool(name="sec", bufs=2) as spool, \
                 tc.tile_pool(name="gath", bufs=3) as gpool:
                qcount = 0
                for pr in range(2):
                    table = table1 if pr == 0 else table2
                    go = so = 0
                    for s in range(16):
                        r, q = divmod(s, 4)
                        Lc = commonL[s]
                        stok, ssc = sec_tok[s], sec_scat[s]
                        if stok == 0:
                            go += stok
                            so += ssc
                            continue
                        gi_b = spool.tile([P, max_tok // 16], I16, tag="gi")
                        si_b = spool.tile([P, max_scat // 16], I16, tag="si")
                        for k in range(8):
                            nc.sync.dma_start(
                                out=gi_b[16 * k:16 * k + 16, :stok // 16],
                                in_=gidx_d[:, go // 16:(go + stok) // 16])
                            nc.sync.dma_start(
                                out=si_b[16 * k:16 * k + 16, :ssc // 16],
                                in_=sidx_d[:, so // 16:(so + ssc) // 16])
                        wbf = spool.tile([P, max_tok // P], BF16, tag="wbf")
                        nc.sync.dma_start(out=wbf[:, :stok // P],
                                          in_=gw_d[:, go // P:(go + stok) // P])
                        w_b = spool.tile([P, max_tok // P], F32, tag="wf")
                        nc.vector.tensor_copy(out=w_b[:, :stok // P],
                                              in_=wbf[:, :stok // P])

                        tbl = table[q * QN:(q + 1) * QN, :]
                        accr = acc[pr][r * QN:(r + 1) * QN, :]
                        lgo = lso = 0   # local token / scatter offsets
                        b = 0
                        while b < len(Lc):
                            L = int(Lc[b])
                            b2 = b
                            while b2 < len(Lc) and int(Lc[b2]) == L:
                                b2 += 1
                            if L == 0:
                                b = b2
                                continue
                            assert L <= 64, L
                            gpc = max(1, 64 // L)
                            bb = b
                            while bb < b2:
                                nbb = min(gpc, b2 - bb)
                                ncols = nbb * L
                                tok = ncols * P
                                stk = nbb * P
                                pk = gpool.tile([P, 64, H], F32, tag="pk")
                                if L == 1:
                                    gt = gpool.tile([P, 64, H], F32, tag="g")
                                    nc.gpsimd.dma_gather(
                                        out_ap=gt[:, :ncols, :], in_ap=tbl,
                                        idxs_ap=gi_b[:, lgo // 16:(lgo + tok) // 16],
                                        num_idxs=tok, num_idxs_reg=tok,
                                        elem_size=H, single_packet=False,
                                        queue_num=qcount % 4)
                                    nc.vector.tensor_tensor(
                                        out=pk[:, :ncols, :], in0=gt[:, :ncols, :],
                                        in1=w_b[:, lgo // P:lgo // P + ncols]
                                            .unsqueeze(-1)
                                            .broadcast_to([P, ncols, H]),
                                        op=mybir.AluOpType.mult)
                                else:
                                    gt = gpool.tile([P, 64, H], F32, tag="g")
                                    nc.gpsimd.dma_gather(
                                        out_ap=gt[:, :ncols, :], in_ap=tbl,
                                        idxs_ap=gi_b[:, lgo // 16:(lgo + tok) // 16],
                                        num_idxs=tok, num_idxs_reg=tok,
                                        elem_size=H, single_packet=False,
                                        queue_num=qcount % 4)
                                    nc.vector.tensor_tensor(
                                        out=gt[:, :ncols, :], in0=gt[:, :ncols, :],
                                        in1=w_b[:, lgo // P:lgo // P + ncols]
                                            .unsqueeze(-1)
                                            .broadcast_to([P, ncols, H]),
                                        op=mybir.AluOpType.mult)
                                    gv = gt[:, :ncols, :].rearrange(
                                        "p (g l) h -> p g l h", l=L)
                                    width = L
                                    while width > 2:
                                        half = width // 2
                                        nc.vector.tensor_tensor(
                                            out=gv[:, :, :half, :],
                                            in0=gv[:, :, :half, :],
                                            in1=gv[:, :, width - half:width, :],
                                            op=mybir.AluOpType.add)
                                        width -= half
                                    if width == 2:
                                        nc.vector.tensor_tensor(
                                            out=pk[:, :nbb, :],
                                            in0=gv[:, :, 0, :], in1=gv[:, :, 1, :],
                                            op=mybir.AluOpType.add)
                                    else:
                                        nc.vector.tensor_copy(
                                            out=pk[:, :nbb, :], in_=gv[:, :, 0, :])
                                nc.gpsimd.dma_scatter_add(
                                    accr, pk[:, :nbb, :],
                                    si_b[:, lso // 16:(lso + stk) // 16],
                                    stk, stk, H, queue_num=qcount % 4)
                                qcount += 1
                                lgo += tok
                                lso += stk
                                bb += nbb
                            b = b2
                        go += stok
                        so += ssc

                    # ---- finish pass ----
                    if pr == 0:
                        with tc.tile_pool(name="fin", bufs=2) as fpool:
                            for k in range(0, NBLK, 49):
                                av = fpool.tile([P, 49, H], F32, tag="av")
                                nc.sync.dma_start(
                                    out=av[:],
                                    in_=acc[0][k * P:(k + 49) * P, :].rearrange(
                                        "(j p) h -> p j h", p=P))
                                nc.vector.tensor_tensor(
                                    out=av[:], in0=av[:],
                                    in1=b1_t[:].unsqueeze(1)
                                        .broadcast_to([P, 49, H]),
                                    op=mybir.AluOpType.add)
                                nc.vector.tensor_scalar_max(
                                    out=av[:], in0=av[:], scalar1=0.0)
                                nc.vector.tensor_tensor(
                                    out=av[:], in0=av[:],
                                    in1=dinv_t[:, k:k + 49].unsqueeze(-1)
                                        .broadcast_to([P, 49, H]),
                                    op=mybir.AluOpType.mult)
                                nc.sync.dma_start(
                                    out=table2[k * P:(k + 49) * P, :].rearrange(
                                        "(j p) h -> p j h", p=P),
                                    in_=av[:])

            # ---- finish2: W2, bias, tanh, transpose to feature-major ----
            with tc.tile_pool(name="f2", bufs=3) as f2pool, \
                 tc.tile_pool(name="f2p", bufs=4, space="PSUM") as f2ps:
                for j in range(NCORES):          # peer slab
                    for c0 in range(0, 98, 8):
                        nb2 = min(8, 98 - c0)
                        k0 = j * 98 + c0
                        av2 = f2pool.tile([P, 8, H], F32, tag="av2")
                        nc.sync.dma_start(
                            out=av2[:, :nb2, :],
                            in_=acc[1][k0 * P:(k0 + nb2) * P, :].rearrange(
                                "(j p) h -> p j h", p=P))
                        for g0 in range(0, nb2, 4):
                            ng = min(4, nb2 - g0)
                            eg = f2pool.tile([H, 4 * P], BF16, tag="eg")
                            pt4 = f2ps.tile([H, 4 * P], F32, tag="pt")
                            for bi in range(ng):
                                nc.tensor.transpose(
                                    out=pt4[:, bi * P:(bi + 1) * P],
                                    in_=av2[:, g0 + bi, :],
                                    identity=ident[:])
                            abT4 = f2pool.tile([H, 4 * P], F32, tag="abT")
                            nc.vector.tensor_copy(out=abT4[:, :ng * P],
                                                  in_=pt4[:, :ng * P])
                            mm4 = f2ps.tile([H, 4 * P], F32, tag="mm")
                            for bi in range(ng):
                                nc.tensor.matmul(
                                    out=mm4[:, bi * P:(bi + 1) * P],
                                    lhsT=W2_t[:],
                                    rhs=abT4[:, bi * P:(bi + 1) * P],
                                    start=True, stop=True)
                            nc.scalar.activation(
                                out=eg[:, :ng * P], in_=mm4[:, :ng * P],
                                func=mybir.ActivationFunctionType.Tanh,
                                bias=b2c_t[:])
                            cz = (c0 + g0) * P
                            nc.sync.dma_start(
                                out=cc_in[j * H:(j + 1) * H, cz:cz + ng * P],
                                in_=eg[:, :ng * P])

            # ---- AllToAll reshard ----
            nc.gpsimd.collective_compute(
                "AllToAll", mybir.AluOpType.bypass,
                replica_groups=[list(range(NCORES))],
                ins=[cc_in[:]], outs=[cc_out[:]])

            # ---- GRU + head (transposed layout) ----
            with tc.tile_pool(name="gs", bufs=1) as gspool, \
                 tc.tile_pool(name="gx", bufs=2) as gxpool, \
                 tc.tile_pool(name="gw", bufs=2) as gwpool, \
                 tc.tile_pool(name="gp", bufs=2, space="PSUM") as gppool, \
                 tc.tile_pool(name="gp1", bufs=1, space="PSUM") as gppool1:
                h32 = gspool.tile([H, NSH], F32)
                nc.gpsimd.memset(h32[:], 0.0)
                for t in range(T):
                    xsT = gxpool.tile([H, NSH], BF16, tag="xs")
                    nc.sync.dma_start(out=xsT[:],
                                      in_=cc_out[t * H:(t + 1) * H, :])
                    y_t = gxpool.tile([Z, NSH], BF16, tag="y")
                    for i in range(NGCH):
                        sl = slice(i * GCH, (i + 1) * GCH)
                        hb = gwpool.tile([H, GCH], BF16, tag="hb")
                        nc.vector.tensor_copy(out=hb[:], in_=h32[:, sl])
                        mm_r = gppool.tile([H, GCH], F32, tag="mr")
                        nc.tensor.matmul(out=mm_r[:], lhsT=wih_t[:, :H],
                                         rhs=xsT[:, sl], start=True, stop=False)
                        nc.tensor.matmul(out=mm_r[:], lhsT=whh_t[:, :H],
                                         rhs=hb[:], start=False, stop=True)
                        mm_z = gppool.tile([H, GCH], F32, tag="mz")
                        nc.tensor.matmul(out=mm_z[:], lhsT=wih_t[:, H:P],
                                         rhs=xsT[:, sl], start=True, stop=False)
                        nc.tensor.matmul(out=mm_z[:], lhsT=whh_t[:, H:P],
                                         rhs=hb[:], start=False, stop=True)
                        r_sb = gwpool.tile([H, GCH], F32, tag="r")
                        nc.scalar.activation(
                            out=r_sb[:], in_=mm_r[:],
                            func=mybir.ActivationFunctionType.Sigmoid,
                            bias=br_t[:])
                        z_sb = gwpool.tile([H, GCH], F32, tag="z")
                        nc.scalar.activation(
                            out=z_sb[:], in_=mm_z[:],
                            func=mybir.ActivationFunctionType.Sigmoid,
                            bias=bz_t[:])
                        mm_hn = gppool1.tile([H, GCH], F32, tag="mhn")
                        nc.tensor.matmul(out=mm_hn[:], lhsT=whh_t[:, P:],
                                         rhs=hb[:], start=True, stop=True)
                        rn = gwpool.tile([H, GCH], F32, tag="rn")
                        nc.vector.tensor_scalar_add(
                            out=rn[:], in0=mm_hn[:], scalar1=bhn_t[:])
                        nc.vector.tensor_tensor(
                            out=rn[:], in0=rn[:], in1=r_sb[:],
                            op=mybir.AluOpType.mult)
                        mm_in = gppool1.tile([H, GCH], F32, tag="min")
                        nc.tensor.matmul(out=mm_in[:], lhsT=wih_t[:, P:],
                                         rhs=xsT[:, sl], start=True, stop=True)
                        npre = gwpool.tile([H, GCH], F32, tag="npre")
                        nc.vector.tensor_tensor(
                            out=npre[:], in0=mm_in[:], in1=rn[:],
                            op=mybir.AluOpType.add)
                        n_sb = gwpool.tile([H, GCH], F32, tag="nsb")
                        nc.scalar.activation(
                            out=n_sb[:], in_=npre[:],
                            func=mybir.ActivationFunctionType.Tanh,
                            bias=bin_t[:])
                        d = gwpool.tile([H, GCH], F32, tag="d")
                        nc.vector.tensor_tensor(
                            out=d[:], in0=h32[:, sl], in1=n_sb[:],
                            op=mybir.AluOpType.subtract)
                        nc.vector.tensor_tensor(
                            out=d[:], in0=d[:], in1=z_sb[:],
                            op=mybir.AluOpType.mult)
                        nc.vector.tensor_tensor(
                            out=h32[:, sl], in0=n_sb[:], in1=d[:],
                            op=mybir.AluOpType.add)
                        hb2 = gwpool.tile([H, GCH], BF16, tag="hb2")
                        nc.vector.tensor_copy(out=hb2[:], in_=h32[:, sl])
                        mm_y = gppool.tile([Z, GCH], F32, tag="my")
                        nc.tensor.matmul(out=mm_y[:], lhsT=wlin_t[:],
                                         rhs=hb2[:], start=True, stop=True)
                        nc.vector.tensor_scalar_add(
                            out=y_t[:, sl], in0=mm_y[:], scalar1=blin_t[:])
                    nc.sync.dma_start(out=ysT_d[t], in_=y_t[:])
    nc.compile()
    return nc


def _warm_devices():
    try:
        import jax
        from jax.sharding import Mesh, PartitionSpec, NamedSharding
        devs = jax.devices()[:NCORES]
        mesh = Mesh(np.asarray(devs), ("core",))
        sh = NamedSharding(mesh, PartitionSpec("core"))
        jax.device_put(np.zeros((NCORES, 4), np.float32), sh).block_until_ready()
    except Exception as e:
        print(f"[kernel] device warm-up failed: {e}", file=sys.stderr)


def _start_warm():
    try:
        import threading
        import jax
        jax.devices()      # backend init on the importing thread
        th = threading.Thread(target=_warm_devices, daemon=True)
        th.start()
        return th
    except Exception as e:
        print(f"[kernel] warm start failed: {e}", file=sys.stderr)
        return None


_WARM = _start_warm()


def kernel(**inputs):
    warm = _WARM if _WARM is not None else _start_warm()
    x = np.asarray(inputs["x"], np.float32)
    eis = np.asarray(inputs["eis"])
    ews = np.asarray(inputs["ews"], np.float32)
    W1 = np.asarray(inputs["W1"], np.float32)
    b1 = np.asarray(inputs["b1"], np.float32)
    W2 = np.asarray(inputs["W2"], np.float32)
    b2 = np.asarray(inputs["b2"], np.float32)
    Wih = np.asarray(inputs["Wih"], np.float32)
    Whh = np.asarray(inputs["Whh"], np.float32)
    bih = np.asarray(inputs["bih"], np.float32)
    bhh = np.asarray(inputs["bhh"], np.float32)
    Wlin = np.asarray(inputs["Wlin"], np.float32)
    blin = np.asarray(inputs["blin"], np.float32)

    _t0 = time.time()
    hp = build_host(x, eis, ews, W1)
    _t0 = _tick("host-prep", _t0)

    nc = build_program(hp)
    _t0 = _tick("build", _t0)

    b1b = np.broadcast_to(b1, (P, H)).copy()
    b2c = b2.reshape(H, 1).copy()
    brc = (bih[:H] + bhh[:H]).reshape(H, 1).copy()
    bzc = (bih[H:2 * H] + bhh[H:2 * H]).reshape(H, 1).copy()
    binc = bih[2 * H:].reshape(H, 1).copy()
    bhnc = bhh[2 * H:].reshape(H, 1).copy()
    blinc = blin.reshape(Z, 1).copy()
    wihT = np.ascontiguousarray(Wih.T).astype(BF)
    whhT = np.ascontiguousarray(Whh.T).astype(BF)
    wlinT = np.ascontiguousarray(Wlin.T).astype(BF)

    in_maps = []
    for c in range(NCORES):
        st = hp["streams"][c]
        tb = hp["tables"][c]
        in_maps.append({
            "xw1bf": hp["xw1bf"],
            "gidx16": wrap16(st["gidx"]),
            "gw128": np.ascontiguousarray(
                st["gw"].reshape(-1, P).T).astype(BF),
            "sidx16": wrap16(st["sidx"]),
            "dinv_blk": tb["dinv_blk"],
            "b1b": b1b, "b2c": b2c, "W2": W2,
            "wihT": wihT, "whhT": whhT, "wlinT": wlinT,
            "br": brc, "bz": bzc, "bin": binc, "bhn": bhnc, "blin": blinc,
        })
    _t0 = _tick("inmaps", _t0)
    if warm is not None:
        warm.join()
    _t0 = _tick("warm-join", _t0)

    res = run_bass_kernel_spmd(nc, in_maps, core_ids=list(range(NCORES)))
    _t0 = _tick("run", _t0)

    out = np.empty((T, N, Z), np.float32)
    for c in range(NCORES):
        lo, hi = c * NSH, min((c + 1) * NSH, N)
        if lo >= N:
            continue
        ys = np.asarray(res.results[c]["ysT"], dtype=np.float32)  # [T, Z, NSH]
        out[:, lo:hi, :] = ys.transpose(0, 2, 1)[:, :hi - lo, :]
    _t0 = _tick("assemble", _t0)
    return out
